# revision 9
# baseline (speedup 1.0000x reference)
"""DFEM kernel for 8 TRN2 NeuronCores.

Data-parallel over batch B=8: core b computes sample b end-to-end
(conv1x1 -> spatial-attention weight, PAM self-attention on both inputs,
final combine). No collectives.

Shapes (hardcoded): B=8, C=256, C8=32, H=W=64, N=4096.

Attention is computed transposed: energy^T chunks [j,i] = k_chunk^T @ q,
exp on ScalarE (logits are tiny, no max subtraction needed), softmax
denominator Z[i] via ones-vector matmul on TensorE, PV via v^T (computed
directly in transposed layout), normalization folded into the epilogue.
"""

import numpy as np
import ml_dtypes

BF16 = ml_dtypes.bfloat16

B, C, C8, H, W = 8, 256, 32, 64, 64
N = H * W          # 4096
P = 128            # partitions
NCT = C // P       # 2 c-tiles
NB = 512           # i-block size
NIB = N // NB      # 8 i-blocks
JB = 128           # j-chunk size
NJT = N // JB      # 32 j-chunks

_CACHE = {}


def _build_program():
    import concourse.bacc as bacc
    import concourse.mybir as mybir
    import concourse.tile as tile

    f32 = mybir.dt.float32
    bf16 = mybir.dt.bfloat16
    AF = mybir.ActivationFunctionType
    ALU = mybir.AluOpType

    nc = bacc.Bacc("TRN2", target_bir_lowering=False, debug=False, num_devices=B)

    # ---- DRAM I/O ----
    x1f = nc.dram_tensor("x1f", (C, N), f32, kind="ExternalInput")
    x1b = nc.dram_tensor("x1b", (C, N), bf16, kind="ExternalInput")
    x2f = nc.dram_tensor("x2f", (C, N), f32, kind="ExternalInput")
    x2b = nc.dram_tensor("x2b", (C, N), bf16, kind="ExternalInput")
    w1T = nc.dram_tensor("w1T", (C, C), bf16, kind="ExternalInput")
    wqT = nc.dram_tensor("wqT", (C, C8), bf16, kind="ExternalInput")
    wkT = nc.dram_tensor("wkT", (C, C8), bf16, kind="ExternalInput")
    wvT = nc.dram_tensor("wvT", (C, C), bf16, kind="ExternalInput")
    b1c = nc.dram_tensor("b1c", (C, 1), f32, kind="ExternalInput")
    bqc = nc.dram_tensor("bqc", (C8, 1), f32, kind="ExternalInput")
    bkc = nc.dram_tensor("bkc", (C8, 1), f32, kind="ExternalInput")
    bv_rep = nc.dram_tensor("bv_rep", (P, C), f32, kind="ExternalInput")
    gamma_s = nc.dram_tensor("gamma_s", (1, 1), f32, kind="ExternalInput")
    wsa_rep = nc.dram_tensor("wsa_rep", (64, 18), f32, kind="ExternalInput")
    ones_c = nc.dram_tensor("ones_c", (P, 1), bf16, kind="ExternalInput")
    out_d = nc.dram_tensor("out", (C, N), f32, kind="ExternalOutput")

    # scratch in DRAM for the f32 conv outputs (residual source)
    x11_d = nc.dram_tensor("x11_scratch", (C, N), f32, kind="Internal")
    x21_d = nc.dram_tensor("x21_scratch", (C, N), f32, kind="Internal")

    def ct_tiles(ap):  # [C, N] -> [2, 128, N]
        return ap.rearrange("(t p) n -> t p n", p=P)

    x1f_t, x1b_t = ct_tiles(x1f), ct_tiles(x1b)
    x2f_t, x2b_t = ct_tiles(x2f), ct_tiles(x2b)
    w1T_t, wvT_t = ct_tiles(w1T), ct_tiles(wvT)
    wqT_t, wkT_t = ct_tiles(wqT), ct_tiles(wkT)
    b1c_t = b1c.rearrange("(t p) o -> t p o", p=P)
    x11_dt, x21_dt = ct_tiles(x11_d), ct_tiles(x21_d)
    out_dt = ct_tiles(out_d)

    with tile.TileContext(nc) as tc:
        from contextlib import ExitStack
        with ExitStack() as ctx:
            consts = ctx.enter_context(tc.tile_pool(name="consts", bufs=1))
            persist = ctx.enter_context(tc.tile_pool(name="persist", bufs=1))
            stream = ctx.enter_context(tc.tile_pool(name="stream", bufs=2))
            apool = ctx.enter_context(tc.tile_pool(name="apool", bufs=6))
            ps512 = ctx.enter_context(tc.tile_pool(name="ps512", bufs=4, space="PSUM"))
            pvps = ctx.enter_context(tc.tile_pool(name="pvps", bufs=2, space="PSUM"))
            zps = ctx.enter_context(tc.tile_pool(name="zps", bufs=1, space="PSUM"))

            # ---- load constants ----
            def cload(ap, shape, dtype, tag):
                t = consts.tile(shape, dtype, tag=tag, name=tag)
                nc.sync.dma_start(out=t, in_=ap)
                return t

            w1T_s = [cload(w1T_t[i], [P, C], bf16, f"w1T{i}") for i in range(NCT)]
            wqT_s = [cload(wqT_t[i], [P, C8], bf16, f"wqT{i}") for i in range(NCT)]
            wkT_s = [cload(wkT_t[i], [P, C8], bf16, f"wkT{i}") for i in range(NCT)]
            wvT_s = [cload(wvT_t[i], [P, C], bf16, f"wvT{i}") for i in range(NCT)]
            b1_s = [cload(b1c_t[i], [P, 1], f32, f"b1{i}") for i in range(NCT)]
            bq_s = cload(bqc[:, :], [C8, 1], f32, "bq")
            bk_s = cload(bkc[:, :], [C8, 1], f32, "bk")
            bv_s = cload(bv_rep[:, :], [P, C], f32, "bv")
            gam_s = cload(gamma_s[:, :], [1, 1], f32, "gam")
            wsa_s = cload(wsa_rep[:, :], [64, 18], f32, "wsa")
            ones_s = cload(ones_c[:, :], [P, 1], bf16, "ones")

            # ---- persistent tiles ----
            x11b = [persist.tile([P, N], bf16, tag=f"x11b{i}", name=f"x11b{i}") for i in range(NCT)]
            x21b = [persist.tile([P, N], bf16, tag=f"x21b{i}", name=f"x21b{i}") for i in range(NCT)]
            q_sb = persist.tile([C8, N], bf16, tag="q_sb", name="q_sb")
            k_sb = persist.tile([C8, N], bf16, tag="k_sb", name="k_sb")
            vT_sb = persist.tile([P, NJT * C], bf16, tag="vT_sb", name="vT_sb")
            out1 = [persist.tile([P, N], f32, tag=f"out1_{i}", name=f"out1_{i}") for i in range(NCT)]
            out2 = [persist.tile([P, N], f32, tag=f"out2_{i}", name=f"out2_{i}") for i in range(NCT)]
            zg_rep = persist.tile([P, N], f32, tag="zg_rep", name="zg_rep")
            # 3 dy-shifted padded planes per channel: plane[ky][h, 1+w] holds
            # image row h+ky-1 (zeros outside). Taps then always read
            # partition base 0 (DVE requires 32-aligned partition offsets).
            planes = [[persist.tile([64, 66], f32, tag=f"plane{c}{k}",
                                    name=f"plane{c}{k}")
                       for k in range(3)] for c in range(2)]
            acc_sa = persist.tile([64, 64], f32, tag="acc_sa", name="acc_sa")
            w64 = persist.tile([64, 64], f32, tag="w64", name="w64")

            # ================= conv1x1 (shared weights) =================
            def conv(xb_dram_t, xout_b, xf_dram_t, fstore_t):
                # load bf16 input tiles
                xin = []
                for i in range(NCT):
                    t = stream.tile([P, N], bf16, tag="stream", name="stream")
                    nc.sync.dma_start(out=t, in_=xb_dram_t[i])
                    xin.append(t)
                for ot in range(NCT):
                    for nb in range(NIB):
                        ps = ps512.tile([P, NB], f32, tag="ps512", name="ps512")
                        sl = slice(nb * NB, (nb + 1) * NB)
                        nc.tensor.matmul(ps, w1T_s[0][:, ot * P:(ot + 1) * P],
                                         xin[0][:, sl], start=True, stop=False)
                        nc.tensor.matmul(ps, w1T_s[1][:, ot * P:(ot + 1) * P],
                                         xin[1][:, sl], start=False, stop=True)
                        # biased bf16 copy for downstream matmuls / SA
                        nc.scalar.activation(xout_b[ot][:, sl], ps, AF.Identity,
                                             bias=b1_s[ot][:, 0:1])
                        # un-biased f32 to scratch tile (bias re-applied at residual)
                        nc.scalar.activation(fstore_t[ot][:, sl], ps, AF.Copy)
                for i in range(NCT):
                    nc.sync.dma_start(out=xf_dram_t[i], in_=fstore_t[i])

            # conv1 stores f32 via out1 tiles, conv2 via out2 tiles
            conv(x1b_t, x11b, x11_dt, out1)
            conv(x2b_t, x21b, x21_dt, out2)

            # ================= spatial attention weight ==================
            # mean over 512 channels via ones-matmul (scaled by 1/512)
            for nb in range(NIB):
                sl = slice(nb * NB, (nb + 1) * NB)
                mp = zps.tile([1, NB], f32, tag="zps", name="zps")
                first = True
                for src in (x11b[0], x11b[1], x21b[0], x21b[1]):
                    nc.tensor.matmul(mp, ones_s, src[:, sl],
                                     start=first, stop=(src is x21b[1]))
                    first = False
                nc.scalar.activation(out2[0][0:1, sl], mp[0:1, :], AF.Identity,
                                     scale=1.0 / (2 * C))
            # max over 512 channels: pairwise DVE max then partition all-reduce
            nc.vector.tensor_tensor(out2[1], x11b[0], x11b[1], op=ALU.max)
            nc.vector.tensor_tensor(out2[1], out2[1], x21b[0], op=ALU.max)
            nc.vector.tensor_tensor(out2[1], out2[1], x21b[1], op=ALU.max)
            import concourse.bass_isa as bass_isa
            nc.gpsimd.partition_all_reduce(out1[0], out2[1], channels=P,
                                           reduce_op=bass_isa.ReduceOp.max)

            # 3x3 conv (2->1 ch) + sigmoid on the 64x64 grid
            for ci, row in ((0, out2[0]), (1, out1[0])):
                img = row[0:1, 0:N].rearrange("p (h w) -> p h w", h=64)
                for ky in range(3):
                    pl = planes[ci][ky]
                    nc.vector.memset(pl, 0.0)
                    if ky == 0:    # plane rows 1..63 <- image rows 0..62
                        nc.sync.dma_start(out=pl[1:64, 1:65], in_=img[:, 0:63, :])
                    elif ky == 1:  # plane rows 0..63 <- image rows 0..63
                        nc.sync.dma_start(out=pl[0:64, 1:65], in_=img[:, 0:64, :])
                    else:          # plane rows 0..62 <- image rows 1..63
                        nc.sync.dma_start(out=pl[0:63, 1:65], in_=img[:, 1:64, :])
            tap = 0
            for ci in range(2):
                for ky in range(3):
                    for kx in range(3):
                        wcol = wsa_s[0:64, tap:tap + 1]
                        window = planes[ci][ky][0:64, kx:kx + 64]
                        if tap == 0:
                            nc.vector.tensor_scalar_mul(acc_sa, window, wcol)
                        else:
                            nc.vector.scalar_tensor_tensor(
                                acc_sa, window, wcol, acc_sa,
                                op0=ALU.mult, op1=ALU.add)
                        tap += 1
            nc.scalar.activation(w64, acc_sa, AF.Sigmoid)

            # ================= PAM attention (one input path) ============
            def qkv(xb):
                for nb in range(NIB):
                    sl = slice(nb * NB, (nb + 1) * NB)
                    for dst, wT, bias in ((q_sb, wqT_s, bq_s), (k_sb, wkT_s, bk_s)):
                        ps = ps512.tile([C8, NB], f32, tag="ps512", name="ps512")
                        nc.tensor.matmul(ps, wT[0], xb[0][:, sl], start=True, stop=False)
                        nc.tensor.matmul(ps, wT[1], xb[1][:, sl], start=False, stop=True)
                        nc.scalar.activation(dst[:, sl], ps, AF.Identity,
                                             bias=bias[:, 0:1])
                for jt in range(NJT):
                    jsl = slice(jt * JB, (jt + 1) * JB)
                    ps = pvps.tile([P, NB], f32, tag="pvps", name="pvps")
                    nc.tensor.matmul(ps[:, 0:C], xb[0][:, jsl], wvT_s[0],
                                     start=True, stop=False)
                    nc.tensor.matmul(ps[:, 0:C], xb[1][:, jsl], wvT_s[1],
                                     start=False, stop=True)
                    nc.vector.tensor_tensor(
                        vT_sb[:, jt * C:(jt + 1) * C], ps[:, 0:C], bv_s, op=ALU.add)

            def attention(outp):
                """energy^T/exp/Z/PV pipeline; writes unnormalized PV into outp
                tiles and 1/Z into zg_rep row 0."""
                for ib in range(NIB):
                    isl = slice(ib * NB, (ib + 1) * NB)
                    pv = [pvps.tile([P, NB], f32, tag="pvps", name="pvps") for _ in range(NCT)]
                    zp = zps.tile([1, NB], f32, tag="zps", name="zps")
                    etiles = {}
                    ats = {}

                    def consume(jt):
                        at = apool.tile([P, NB], bf16, tag="apool", name="apool")
                        nc.scalar.activation(at, etiles.pop(jt), AF.Exp)
                        for h in range(NCT):
                            nc.tensor.matmul(
                                pv[h], vT_sb[:, jt * C + h * P: jt * C + (h + 1) * P],
                                at, start=(jt == 0), stop=(jt == NJT - 1),
                                skip_group_check=True)
                        nc.tensor.matmul(zp, ones_s, at,
                                         start=(jt == 0), stop=(jt == NJT - 1),
                                         skip_group_check=True)

                    for jt in range(NJT):
                        ep = ps512.tile([P, NB], f32, tag="ps512", name="ps512")
                        nc.tensor.matmul(ep, k_sb[:, jt * JB:(jt + 1) * JB],
                                         q_sb[:, isl], start=True, stop=True,
                                         skip_group_check=True)
                        etiles[jt] = ep
                        if jt >= 2:
                            consume(jt - 2)
                    consume(NJT - 2)
                    consume(NJT - 1)

                    nc.vector.reciprocal(zg_rep[0:1, isl], zp[0:1, :])
                    for h in range(NCT):
                        nc.scalar.activation(outp[h][:, isl], pv[h], AF.Copy)

            def epilogue(outp, xf_dram_t):
                # zg row: *gamma, broadcast to 128 partitions
                nc.vector.tensor_scalar_mul(zg_rep[0:1, :], zg_rep[0:1, :],
                                            gam_s[0:1, 0:1])
                nc.gpsimd.partition_broadcast(zg_rep, zg_rep[0:1, :])
                for t in range(NCT):
                    st = stream.tile([P, N], f32, tag="stream", name="stream")
                    nc.sync.dma_start(out=st, in_=xf_dram_t[t])
                    nc.vector.tensor_tensor(outp[t], outp[t], zg_rep, op=ALU.mult)
                    # out = (x11_nobias + b1) + gamma*pam/Z
                    nc.vector.scalar_tensor_tensor(
                        outp[t], st, b1_s[t][:, 0:1], outp[t],
                        op0=ALU.add, op1=ALU.add)

            qkv(x11b)
            attention(out1)
            epilogue(out1, x11_dt)

            qkv(x21b)
            attention(out2)
            epilogue(out2, x21_dt)

            # ================= final combine =============================
            # reuse zg_rep as the broadcast spatial-attention weight
            nc.sync.dma_start(out=zg_rep[0:1, 0:N], in_=w64[0:64, 0:64])
            nc.gpsimd.partition_broadcast(zg_rep, zg_rep[0:1, :])
            for t in range(NCT):
                a = stream.tile([P, N], f32, tag="stream", name="stream")
                b = stream.tile([P, N], f32, tag="stream", name="stream")
                nc.sync.dma_start(out=a, in_=x1f_t[t])
                nc.sync.dma_start(out=b, in_=x2f_t[t])
                nc.vector.tensor_tensor(out1[t], out1[t], a, op=ALU.mult)
                nc.vector.tensor_tensor(out2[t], out2[t], b, op=ALU.mult)
                nc.vector.tensor_tensor(out1[t], out2[t], out1[t], op=ALU.subtract)
                nc.scalar.activation(out1[t], out1[t], AF.Abs)
                nc.vector.tensor_tensor(out1[t], out1[t], zg_rep, op=ALU.mult)
                nc.sync.dma_start(out=out_dt[t], in_=out1[t])

    nc.compile()
    return nc


def _prep_inputs(x1, x2, w1, b1, wq, bq, wk, bk, wv, bv, gamma, w_sa):
    shared = {
        "w1T": np.ascontiguousarray(w1.T).astype(BF16),
        "wqT": np.ascontiguousarray(wq.T).astype(BF16),
        "wkT": np.ascontiguousarray(wk.T).astype(BF16),
        "wvT": np.ascontiguousarray(wv.T).astype(BF16),
        "b1c": np.ascontiguousarray(b1.reshape(C, 1)).astype(np.float32),
        "bqc": np.ascontiguousarray(bq.reshape(C8, 1)).astype(np.float32),
        "bkc": np.ascontiguousarray(bk.reshape(C8, 1)).astype(np.float32),
        "bv_rep": np.broadcast_to(bv.reshape(1, C), (P, C)).copy().astype(np.float32),
        "gamma_s": np.asarray(gamma, np.float32).reshape(1, 1).copy(),
        "wsa_rep": np.broadcast_to(
            np.asarray(w_sa, np.float32).reshape(1, 18), (64, 18)).copy(),
        "ones_c": np.ones((P, 1), BF16),
    }
    in_maps = []
    for bidx in range(B):
        x1s = np.ascontiguousarray(x1[bidx].reshape(C, N)).astype(np.float32)
        x2s = np.ascontiguousarray(x2[bidx].reshape(C, N)).astype(np.float32)
        m = dict(shared)
        m["x1f"] = x1s
        m["x1b"] = x1s.astype(BF16)
        m["x2f"] = x2s
        m["x2b"] = x2s.astype(BF16)
        in_maps.append(m)
    return in_maps


def kernel(x1, x2, w1, b1, wq, bq, wk, bk, wv, bv, gamma, w_sa, _trace=False):
    from concourse.bass_utils import run_bass_kernel_spmd

    if "nc" not in _CACHE:
        _CACHE["nc"] = _build_program()
    nc = _CACHE["nc"]

    in_maps = _prep_inputs(np.asarray(x1), np.asarray(x2), np.asarray(w1),
                           np.asarray(b1), np.asarray(wq), np.asarray(bq),
                           np.asarray(wk), np.asarray(bk), np.asarray(wv),
                           np.asarray(bv), np.asarray(gamma), np.asarray(w_sa))
    res = run_bass_kernel_spmd(nc, in_maps, core_ids=list(range(B)), trace=_trace)
    _CACHE["last_result"] = res
    out = np.stack([res.results[c]["out"] for c in range(B)], axis=0)
    return out.reshape(B, C, H, W).astype(np.float32)
